# revision 49
# baseline (speedup 1.0000x reference)
"""Cross-attention layer (B=2, QL=CL=2048, E=1024, 16 heads x 64d) on 8 TRN2 cores.

Sharding: tensor-parallel over heads. Core c owns heads (2c, 2c+1), i.e. a
128-wide feature slice of Wq/Wk/Wv columns and Wo rows. Each core computes a
full-shape partial of the output projection; the host sums the 8 partials and
adds bo.

v4: all math stays bf16 (attention amplifies per-element quantization 1:1 —
fp8 anywhere on the Q/K/V/eg path costs 2-7% output error vs the 2% budget).
Wins over v3 come from the measured DMA model (3 dynamic queues; gpsimd's
swDGE issues don't block on the transfer and its ring is several times
faster than the sync/scalar hw rings once compute runs; ~10 completion sems
recycle per pool in emission order):
  - the whole input stream rides the gpsimd ring as one DMA per logical
    tensor/chunk, in strict deadline order, with only q0/q1 on the sync
    ring; an/ob DMAs get sync's fresh sem pool so they never stall on
    recycled claims.
  - PE warmup is a single accumulation group (back-to-back, no psum-pool
    WAR chain) bridging the framework preamble to the first chunk arrival.
  - attended(u,15) + the unit epilogue are emitted inside unit u+1's ci=0
    slot (software pipeline across unit boundaries); the 8 output-projection
    e-tiles accumulate into one SBUF tile flushed as a single DMA per unit;
    the final unit's psum->sbuf casts alternate DVE/ACT.
Scores use the 2-head row-tiling trick (K=64 pairs execute concurrently in
different PE row groups); softmax skips max-subtraction (scores ~ N(0,1)
after the 1/8 scale) and Z comes from a ones column appended to V.
Steady state is PE-bound (~193us PE busy/core: QKV+O projections 70us,
scores+attended+Z 118us) with ACT exp at ~142us; both run >90% occupancy
mid-kernel.
"""

import numpy as np
import ml_dtypes

E = 1024          # embed dim
H = 16            # heads
D = 64            # head dim
B = 2
QL = CL = 2048
POS = B * QL      # 4096 flattened positions
NCORES = 8
P = 128           # per-core feature slice (2 heads x 64)
ET = E // 128     # 8 contraction e-tiles
NPT = POS // 128  # 32 position tiles (V)
CT = CL // 128    # 16 context tiles per batch
QB = 512          # q-block (free dim of attention matmuls)
NU = POS // QB    # 8 units (b, qb)
VW = 66           # per-head stride in v_sb: 64 V cols + 1 ones + 1 pad

BF16 = ml_dtypes.bfloat16

_CACHE = {}


def _build_nc():
    import concourse.bacc as bacc
    import concourse.mybir as mybir
    import concourse.tile as tile

    bf = mybir.dt.bfloat16
    f32 = mybir.dt.float32
    Exp = mybir.ActivationFunctionType.Exp
    mult = mybir.AluOpType.mult

    nc = bacc.Bacc(
        "TRN2",
        target_bir_lowering=False,
        debug=False,
        enable_asserts=False,
        num_devices=NCORES,
    )

    qT_d = nc.dram_tensor("qT", [E, POS], bf, kind="ExternalInput").ap()
    cT_d = nc.dram_tensor("cT", [E, POS], bf, kind="ExternalInput").ap()
    wq_d = nc.dram_tensor("wq", [E, P], bf, kind="ExternalInput").ap()
    wk_d = nc.dram_tensor("wk", [E, P], bf, kind="ExternalInput").ap()
    wv_d = nc.dram_tensor("wv", [E, P], bf, kind="ExternalInput").ap()
    wo_d = nc.dram_tensor("wo", [P, E], bf, kind="ExternalInput").ap()
    bq_d = nc.dram_tensor("bq", [P, 1], f32, kind="ExternalInput").ap()
    bk_d = nc.dram_tensor("bk", [P, 1], f32, kind="ExternalInput").ap()
    bv_d = nc.dram_tensor("bvt", [128, P], f32, kind="ExternalInput").ap()
    outT_d = nc.dram_tensor("outT", [E, POS], bf, kind="ExternalOutput").ap()
    outT_r = outT_d.rearrange("(t p) m -> p t m", p=128)

    with tile.TileContext(nc) as tc:
        with (
            tc.tile_pool(name="const", bufs=1) as const,
            tc.tile_pool(name="inp", bufs=1) as inp,
            tc.tile_pool(name="proj", bufs=1) as proj,
            tc.tile_pool(name="egp", bufs=3) as egp,
            tc.tile_pool(name="zp", bufs=2) as zp,
            tc.tile_pool(name="anp", bufs=2) as anp,
            tc.tile_pool(name="obp", bufs=2) as obp,
            tc.tile_pool(name="ps_s", bufs=2, space="PSUM") as ps_s,
            tc.tile_pool(name="ps_att", bufs=2, space="PSUM") as ps_att,
            tc.tile_pool(name="ps_m", bufs=2, space="PSUM") as ps_m,
        ):
            # ---- weights needed first, then the first input chunks -------
            wk_sb = const.tile([128, ET, P], bf)
            wq_sb = const.tile([128, ET, P], bf)
            wv_sb = const.tile([128, ET, P], bf)
            bq_sb = const.tile([P, 1], f32)
            bk_sb = const.tile([P, 1], f32)

            qt_sb = inp.tile([128, ET, POS], bf)
            ct_sb = inp.tile([128, ET, POS], bf)
            qT_r = qT_d.rearrange("(t p) m -> p t m", p=128)
            cT_r = cT_d.rearrange("(t p) m -> p t m", p=128)

            def dma_in(eng, which, ch0, ch1, t0=0, t1=ET):
                """Load chunks [ch0, ch1) of qT/cT (e-tiles [t0,t1), one DMA)."""
                src, dst = (qT_r, qt_sb) if which == "q" else (cT_r, ct_sb)
                c0, c1 = ch0 * QB, ch1 * QB
                eng.dma_start(dst[:, t0:t1, c0:c1], src[:, t0:t1, c0:c1])

            wo_sb = const.tile([P, E], bf)
            bv_sb = const.tile([128, P], f32)
            # DMA model: 3 dynamic queues.  gpsimd's (Q0) is the fastest and
            # its issue instructions do NOT block on the transfer (software
            # DGE), while sync/scalar hw queues get starved when Q0 is
            # active and their issues occupy the queue until completion.  So
            # the whole input stream goes on gpsimd, in strict priority
            # order (issue rate ~0.7-1.1us gates the head of the stream).
            # sync keeps its own sem pool free of recycling for the an/ob
            # DMAs; scalar stays pure-ACT.
            g = nc.gpsimd
            g.dma_start(wk_sb[:], wk_d.rearrange("(t p) m -> p t m", p=128))
            dma_in(g, "c", 0, 1, 0, 4)
            dma_in(g, "c", 0, 1, 4, 8)
            g.dma_start(wq_sb[:], wq_d.rearrange("(t p) m -> p t m", p=128))
            g.dma_start(bk_sb[:], bk_d[:])
            g.dma_start(bq_sb[:], bq_d[:])
            g.dma_start(wv_sb[:], wv_d.rearrange("(t p) m -> p t m", p=128))
            g.dma_start(bv_sb[:], bv_d[:])
            # q chunks 0-1 on the sync hw ring, concurrent with gpsimd's
            dma_in(nc.sync, "q", 0, 1)
            dma_in(nc.sync, "q", 1, 2)
            dma_in(nc.scalar, "c", 2, 3)
            dma_in(nc.scalar, "q", 6, 8)
            dma_in(g, "c", 1, 2)
            g.dma_start(wo_sb[:], wo_d[:])
            dma_in(g, "c", 3, 4)
            dma_in(g, "q", 2, 3)
            dma_in(g, "q", 3, 4)
            dma_in(g, "c", 4, 6)
            dma_in(g, "q", 4, 5)
            dma_in(g, "q", 5, 6)
            dma_in(g, "c", 6, 7)
            dma_in(g, "c", 7, 8)

            # row 64 is the lhsT of the K=1 Z-broadcast matmul
            ones65 = const.tile([65, 64], bf)
            nc.vector.memset(ones65[:], 1.0)
            # ACT table warmup: preload EXP during startup (table load ~1.3us)
            warm = const.tile([65, 16], bf)
            nc.scalar.activation(warm[:], ones65[:, 0:16], Exp)

            # ---- projection outputs --------------------------------------
            qproj = proj.tile([P, POS], bf)   # Q^T  (2 heads on partitions)
            kproj = proj.tile([P, POS], bf)   # K^T
            # V position-major: per pos-tile [V_h0(64) | 1 | pad | V_h1(64) | 1 | pad]
            v_sb = proj.tile([128, NPT, 2, VW], bf)
            nc.vector.memset(v_sb[:, :, :, 64:65], 1.0)

            def emit_qk(which, ch):
                """Q^T or K^T projection for one 512-pos chunk."""
                src, w_sb, b_sb, dst = (
                    (qt_sb, wq_sb, bq_sb, qproj)
                    if which == "q"
                    else (ct_sb, wk_sb, bk_sb, kproj)
                )
                c0 = ch * QB
                ps = ps_m.tile([128, QB], f32, tag="m", name=f"psqk{which}{ch}")
                for t in range(ET):
                    nc.tensor.matmul(
                        ps[:],
                        w_sb[:, t, :],
                        src[:, t, c0 : c0 + QB],
                        start=(t == 0),
                        stop=(t == ET - 1),
                    )
                nc.vector.tensor_scalar_add(dst[:, c0 : c0 + QB], ps[:], b_sb[:])

            def emit_v(pt):
                """V projection (position-major) for one 128-pos tile."""
                psv = ps_m.tile([128, 128], f32, tag="m", name=f"psv{pt}")
                for t in range(ET):
                    nc.tensor.matmul(
                        psv[:],
                        ct_sb[:, t, pt * 128 : (pt + 1) * 128],
                        wv_sb[:, t, :],
                        start=(t == 0),
                        stop=(t == ET - 1),
                    )
                nc.vector.tensor_add(
                    v_sb[:, pt, :, 0:64],
                    psv.rearrange("p (h d) -> p h d", h=2),
                    bv_sb[:].rearrange("p (h d) -> p h d", h=2),
                )

            # ---- attention unit machinery --------------------------------
            state = {}

            def unit_start(u):
                state[u] = {
                    "atts": [
                        ps_att.tile([65, QB], f32, tag="att", name=f"att{u}{h}")
                        for h in range(2)
                    ],
                    "eg": {},
                }

            def emit_scores_exp(u, ci):
                b = u // 4
                q0 = u * QB
                c0 = b * CL + ci * 128
                sg = ps_s.tile([128, 2 * QB], f32, tag="sg", name=f"sg{u}_{ci}")
                for h in range(2):
                    hp = h * 64
                    nc.tensor.matmul(
                        sg[:, h * QB : (h + 1) * QB],
                        kproj[hp : hp + 64, c0 : c0 + 128],
                        qproj[hp : hp + 64, q0 : q0 + QB],
                        start=True,
                        stop=True,
                    )
                eg = egp.tile([128, 2 * QB], bf, tag="eg", name=f"eg{u}_{ci}")
                nc.scalar.activation(eg[:], sg[:], Exp, scale=0.125)
                state[u]["eg"][ci] = eg

            def emit_attended(u, ci):
                b = u // 4
                eg = state[u]["eg"].pop(ci)
                for h in range(2):
                    nc.tensor.matmul(
                        state[u]["atts"][h][:],
                        v_sb[:, b * CT + ci, h, 0:65],
                        eg[:, h * QB : (h + 1) * QB],
                        start=(ci == 0),
                        stop=(ci == CT - 1),
                    )

            def emit_epi_copy(u):
                """Evacuate attended+Z psum -> sbuf (frees the atts tiles)."""
                st = state[u]
                st["attus"] = []
                for h in range(2):
                    attu = zp.tile([65, QB], bf, tag=f"attu{h}", name=f"attu{u}{h}")
                    nc.vector.tensor_copy(attu[:], st["atts"][h][:])
                    st["attus"].append(attu)

            def emit_epi_norm(u, h1_first=True):
                """Broadcast Z, normalize -> an tile."""
                st = state[u]
                st["an"] = anp.tile([P, QB], bf, tag="an", name=f"an{u}")
                for h in (1, 0) if h1_first else (0, 1):
                    attu = st["attus"][h]
                    zbp = ps_m.tile([64, QB], f32, tag="m", name=f"zbp{u}{h}")
                    nc.tensor.matmul(
                        zbp[:], ones65[64:65, :], attu[64:65, :], start=True, stop=True
                    )
                    ztr = zp.tile([64, QB], f32, tag=f"ztr{h}", name=f"ztr{u}{h}")
                    nc.vector.reciprocal_approx_fast(ztr[:], zbp[:])
                    if h == 0:
                        nc.vector.tensor_tensor(
                            st["an"][0:64, :], attu[0:64, :], ztr[:], op=mult
                        )
                    else:
                        an1 = zp.tile([64, QB], bf, tag="an1", name=f"an1{u}")
                        nc.vector.tensor_tensor(an1[:], attu[0:64, :], ztr[:], op=mult)
                        nc.sync.dma_start(st["an"][64:128, :], an1[:])

            def emit_epi_po(u, eo, pool_eng=False):
                """One e-tile of the output projection for unit u; the 8
                e-tiles accumulate into one SBUF tile, flushed as one DMA."""
                q0 = u * QB
                st = state[u]
                if eo == 0:
                    st["ob"] = obp.tile([128, ET, QB], bf, tag="ob", name=f"ob{u}")
                po = ps_m.tile([128, QB], f32, tag="m", name=f"po{u}{eo}")
                nc.tensor.matmul(
                    po[:],
                    wo_sb[:, eo * 128 : (eo + 1) * 128],
                    st["an"][:],
                    start=True,
                    stop=True,
                )
                if pool_eng:
                    nc.scalar.copy(st["ob"][:, eo, :], po[:])
                else:
                    nc.vector.tensor_copy(st["ob"][:, eo, :], po[:])
                if u == NU - 1:
                    # tail: halves on the idle gpsimd ring (fast,
                    # non-blocking issue) so the last transfer is small
                    if eo == 3:
                        nc.gpsimd.dma_start(
                            outT_r[:, 0:4, q0 : q0 + QB], st["ob"][:, 0:4, :]
                        )
                    elif eo == ET - 1:
                        nc.gpsimd.dma_start(
                            outT_r[:, 4:ET, q0 : q0 + QB], st["ob"][:, 4:ET, :]
                        )
                elif eo == ET - 1:
                    nc.sync.dma_start(
                        outT_r[:, :, q0 : q0 + QB], st["ob"][:, :, :]
                    )

            # ---- PE p-state warmup: back-to-back accumulating matmuls (no
            # inter-MM deps) while the first input chunks stream in, so real
            # projections start at 2.4GHz --
            warm2 = const.tile([128, QB], bf)
            nc.vector.memset(warm2[:], 0.0)
            wps = ps_m.tile([128, QB], f32, tag="m", name="wps")
            for w in range(20):
                nc.tensor.matmul(
                    wps[:], warm2[:, 0:128], warm2[:], start=(w == 0), stop=(w == 19)
                )

            # ---- pre-unit-0 minimal projections --------------------------
            emit_qk("c", 0)      # kproj ctx chunk 0 (b0)
            emit_qk("q", 0)      # qproj q chunk 0 (unit 0)
            emit_v(0)
            emit_v(1)

            # just-in-time extras: extras[u][ci] emitted after that ci's
            # scores/exp/attended and any epilogue piece. Deadlines: kproj
            # ch c before its first consuming ci; vproj pt before its
            # attended (emitted at ci = pt%16 + 1); qproj u+1 before u+1 ci0.
            extras = {u: {ci: [] for ci in range(CT)} for u in range(NU)}

            def sched(u, ci, fn, *a):
                extras[u][ci].append((fn, a))

            # unit 0: rest of b0 K/V proj
            sched(0, 0, emit_v, 2)
            sched(0, 1, emit_v, 3)
            sched(0, 1, emit_qk, "c", 1)
            sched(0, 2, emit_v, 4)
            sched(0, 3, emit_v, 5)
            sched(0, 4, emit_v, 6)
            sched(0, 5, emit_v, 7)
            sched(0, 5, emit_qk, "c", 2)
            sched(0, 6, emit_v, 8)
            sched(0, 7, emit_v, 9)
            sched(0, 8, emit_v, 10)
            sched(0, 9, emit_v, 11)
            sched(0, 9, emit_qk, "c", 3)
            sched(0, 10, emit_v, 12)
            sched(0, 11, emit_v, 13)
            sched(0, 12, emit_v, 14)
            sched(0, 13, emit_v, 15)
            sched(0, 14, emit_qk, "q", 1)
            # unit 1: qproj for unit 2
            sched(1, 13, emit_qk, "q", 2)
            # unit 2: start b1 context work + qproj(3)
            sched(2, 10, emit_qk, "c", 4)
            sched(2, 11, emit_v, 16)
            sched(2, 12, emit_v, 17)
            sched(2, 13, emit_qk, "q", 3)
            sched(2, 14, emit_v, 18)
            sched(2, 15, emit_v, 19)
            # unit 3: more b1 + qproj(4)
            sched(3, 10, emit_qk, "c", 5)
            sched(3, 11, emit_v, 20)
            sched(3, 12, emit_v, 21)
            sched(3, 13, emit_qk, "c", 6)
            sched(3, 14, emit_v, 22)
            sched(3, 15, emit_qk, "q", 4)
            # unit 4 (b1): remaining b1 vproj just-in-time
            sched(4, 2, emit_v, 23)
            sched(4, 4, emit_v, 24)
            sched(4, 4, emit_qk, "c", 7)
            sched(4, 5, emit_v, 25)
            sched(4, 6, emit_v, 26)
            sched(4, 7, emit_v, 27)
            sched(4, 8, emit_v, 28)
            sched(4, 9, emit_v, 29)
            sched(4, 10, emit_qk, "q", 5)
            sched(4, 11, emit_v, 30)
            sched(4, 12, emit_v, 31)
            sched(5, 10, emit_qk, "q", 6)
            sched(6, 10, emit_qk, "q", 7)

            # ---- main loop: 8 units, software-pipelined across unit
            # boundaries (attended(u-1,15) + epilogue emitted inside unit u)
            for u in range(NU):
                unit_start(u)
                for ci in range(CT):
                    emit_scores_exp(u, ci)
                    if ci >= 1:
                        emit_attended(u, ci - 1)
                    elif u > 0:
                        emit_attended(u - 1, CT - 1)
                        emit_epi_copy(u - 1)
                    if u > 0:
                        if ci == 1:
                            emit_epi_norm(u - 1)
                        elif 2 <= ci <= 9:
                            emit_epi_po(u - 1, ci - 2)
                    for fn, a in extras[u][ci]:
                        fn(*a)
            emit_attended(NU - 1, CT - 1)
            emit_epi_copy(NU - 1)
            emit_epi_norm(NU - 1)
            for eo in range(ET):
                emit_epi_po(NU - 1, eo, pool_eng=(eo % 2 == 1))

    nc.compile()
    return nc


def get_nc():
    if "nc" not in _CACHE:
        _CACHE["nc"] = _build_nc()
    return _CACHE["nc"]


def make_in_maps(query, context, Wq, bq, Wk, bk, Wv, bv, Wo, bo):
    qT = np.asarray(query, np.float32).reshape(POS, E).T.astype(BF16)
    cT = np.asarray(context, np.float32).reshape(POS, E).T.astype(BF16)
    in_maps = []
    for c in range(NCORES):
        F = slice(P * c, P * (c + 1))
        in_maps.append(
            {
                "qT": qT,
                "cT": cT,
                "wq": np.ascontiguousarray(Wq[:, F]).astype(BF16),
                "wk": np.ascontiguousarray(Wk[:, F]).astype(BF16),
                "wv": np.ascontiguousarray(Wv[:, F]).astype(BF16),
                "wo": np.ascontiguousarray(Wo[F, :]).astype(BF16),
                "bq": np.ascontiguousarray(bq[F]).reshape(P, 1).astype(np.float32),
                "bk": np.ascontiguousarray(bk[F]).reshape(P, 1).astype(np.float32),
                "bvt": np.ascontiguousarray(
                    np.broadcast_to(bv[F], (128, P))
                ).astype(np.float32),
            }
        )
    return in_maps


def assemble_output(partials, bo):
    total = np.zeros((E, POS), np.float32)
    for p in partials:
        total += p
    out = total.T.reshape(B, QL, E) + np.asarray(bo, np.float32)
    return out.astype(np.float32)


def kernel(query, context, Wq, bq, Wk, bk, Wv, bv, Wo, bo):
    from concourse import bass_utils

    nc = get_nc()
    in_maps = make_in_maps(query, context, Wq, bq, Wk, bk, Wv, bv, Wo, bo)
    res = bass_utils.run_bass_kernel_spmd(nc, in_maps, core_ids=list(range(NCORES)))
    partials = [res.results[c]["outT"] for c in range(NCORES)]
    return assemble_output(partials, bo)



# revision 50
# speedup vs baseline: 1.0049x; 1.0049x over previous
"""Cross-attention layer (B=2, QL=CL=2048, E=1024, 16 heads x 64d) on 8 TRN2 cores.

Sharding: tensor-parallel over heads. Core c owns heads (2c, 2c+1), i.e. a
128-wide feature slice of Wq/Wk/Wv columns and Wo rows. Each core computes a
full-shape partial of the output projection; the host sums the 8 partials and
adds bo.

v4: all math stays bf16 (attention amplifies per-element quantization 1:1 —
fp8 anywhere on the Q/K/V/eg path costs 2-7% output error vs the 2% budget).
Wins over v3 come from the measured DMA model (3 dynamic queues; gpsimd's
swDGE issues don't block on the transfer and its ring is several times
faster than the sync/scalar hw rings once compute runs; ~10 completion sems
recycle per pool in emission order):
  - the whole input stream rides the gpsimd ring as one DMA per logical
    tensor/chunk, in strict deadline order, with only q0/q1 on the sync
    ring; an/ob DMAs get sync's fresh sem pool so they never stall on
    recycled claims.
  - PE warmup is a single accumulation group (back-to-back, no psum-pool
    WAR chain) bridging the framework preamble to the first chunk arrival.
  - attended(u,15) + the unit epilogue are emitted inside unit u+1's ci=0
    slot (software pipeline across unit boundaries); the 8 output-projection
    e-tiles accumulate into one SBUF tile flushed as a single DMA per unit;
    the final unit's psum->sbuf casts alternate DVE/ACT.
Scores use the 2-head row-tiling trick (K=64 pairs execute concurrently in
different PE row groups); softmax skips max-subtraction (scores ~ N(0,1)
after the 1/8 scale) and Z comes from a ones column appended to V.
Steady state is PE-bound (~193us PE busy/core: QKV+O projections 70us,
scores+attended+Z 118us) with ACT exp at ~142us; both run >90% occupancy
mid-kernel.
"""

import numpy as np
import ml_dtypes

E = 1024          # embed dim
H = 16            # heads
D = 64            # head dim
B = 2
QL = CL = 2048
POS = B * QL      # 4096 flattened positions
NCORES = 8
P = 128           # per-core feature slice (2 heads x 64)
ET = E // 128     # 8 contraction e-tiles
NPT = POS // 128  # 32 position tiles (V)
CT = CL // 128    # 16 context tiles per batch
QB = 512          # q-block (free dim of attention matmuls)
NU = POS // QB    # 8 units (b, qb)
VW = 66           # per-head stride in v_sb: 64 V cols + 1 ones + 1 pad

BF16 = ml_dtypes.bfloat16

_CACHE = {}


def _build_nc():
    import concourse.bacc as bacc
    import concourse.mybir as mybir
    import concourse.tile as tile

    bf = mybir.dt.bfloat16
    f32 = mybir.dt.float32
    Exp = mybir.ActivationFunctionType.Exp
    mult = mybir.AluOpType.mult

    nc = bacc.Bacc(
        "TRN2",
        target_bir_lowering=False,
        debug=False,
        enable_asserts=False,
        num_devices=NCORES,
    )

    qT_d = nc.dram_tensor("qT", [E, POS], bf, kind="ExternalInput").ap()
    cT_d = nc.dram_tensor("cT", [E, POS], bf, kind="ExternalInput").ap()
    wq_d = nc.dram_tensor("wq", [E, P], bf, kind="ExternalInput").ap()
    wk_d = nc.dram_tensor("wk", [E, P], bf, kind="ExternalInput").ap()
    wv_d = nc.dram_tensor("wv", [E, P], bf, kind="ExternalInput").ap()
    wo_d = nc.dram_tensor("wo", [P, E], bf, kind="ExternalInput").ap()
    bq_d = nc.dram_tensor("bq", [P, 1], f32, kind="ExternalInput").ap()
    bk_d = nc.dram_tensor("bk", [P, 1], f32, kind="ExternalInput").ap()
    bv_d = nc.dram_tensor("bvt", [128, P], f32, kind="ExternalInput").ap()
    outT_d = nc.dram_tensor("outT", [E, POS], bf, kind="ExternalOutput").ap()
    outT_r = outT_d.rearrange("(t p) m -> p t m", p=128)

    with tile.TileContext(nc) as tc:
        with (
            tc.tile_pool(name="const", bufs=1) as const,
            tc.tile_pool(name="inp", bufs=1) as inp,
            tc.tile_pool(name="proj", bufs=1) as proj,
            tc.tile_pool(name="egp", bufs=3) as egp,
            tc.tile_pool(name="zp", bufs=2) as zp,
            tc.tile_pool(name="anp", bufs=2) as anp,
            tc.tile_pool(name="obp", bufs=2) as obp,
            tc.tile_pool(name="ps_s", bufs=2, space="PSUM") as ps_s,
            tc.tile_pool(name="ps_att", bufs=2, space="PSUM") as ps_att,
            tc.tile_pool(name="ps_m", bufs=2, space="PSUM") as ps_m,
        ):
            # ---- weights needed first, then the first input chunks -------
            wk_sb = const.tile([128, ET, P], bf)
            wq_sb = const.tile([128, ET, P], bf)
            wv_sb = const.tile([128, ET, P], bf)
            bq_sb = const.tile([P, 1], f32)
            bk_sb = const.tile([P, 1], f32)

            qt_sb = inp.tile([128, ET, POS], bf)
            ct_sb = inp.tile([128, ET, POS], bf)
            qT_r = qT_d.rearrange("(t p) m -> p t m", p=128)
            cT_r = cT_d.rearrange("(t p) m -> p t m", p=128)

            def dma_in(eng, which, ch0, ch1, t0=0, t1=ET):
                """Load chunks [ch0, ch1) of qT/cT (e-tiles [t0,t1), one DMA)."""
                src, dst = (qT_r, qt_sb) if which == "q" else (cT_r, ct_sb)
                c0, c1 = ch0 * QB, ch1 * QB
                eng.dma_start(dst[:, t0:t1, c0:c1], src[:, t0:t1, c0:c1])

            wo_sb = const.tile([P, E], bf)
            bv_sb = const.tile([128, P], f32)
            # DMA model: 3 dynamic queues.  gpsimd's (Q0) is the fastest and
            # its issue instructions do NOT block on the transfer (software
            # DGE), while sync/scalar hw queues get starved when Q0 is
            # active and their issues occupy the queue until completion.  So
            # the whole input stream goes on gpsimd, in strict priority
            # order (issue rate ~0.7-1.1us gates the head of the stream).
            # sync keeps its own sem pool free of recycling for the an/ob
            # DMAs; scalar stays pure-ACT.
            g = nc.gpsimd
            g.dma_start(wk_sb[:], wk_d.rearrange("(t p) m -> p t m", p=128))
            dma_in(g, "c", 0, 1, 0, 4)
            dma_in(g, "c", 0, 1, 4, 8)
            g.dma_start(wq_sb[:], wq_d.rearrange("(t p) m -> p t m", p=128))
            g.dma_start(bk_sb[:], bk_d[:])
            g.dma_start(bq_sb[:], bq_d[:])
            g.dma_start(wv_sb[:], wv_d.rearrange("(t p) m -> p t m", p=128))
            g.dma_start(bv_sb[:], bv_d[:])
            # q chunks 0-1 on the sync hw ring, concurrent with gpsimd's
            dma_in(nc.sync, "q", 0, 1)
            dma_in(nc.sync, "q", 1, 2)
            dma_in(nc.scalar, "c", 2, 3)
            dma_in(nc.scalar, "q", 6, 8)
            dma_in(g, "c", 1, 2)
            g.dma_start(wo_sb[:], wo_d[:])
            dma_in(g, "c", 3, 4)
            dma_in(g, "q", 2, 3)
            dma_in(g, "q", 3, 4)
            dma_in(g, "c", 4, 6)
            dma_in(g, "q", 4, 5)
            dma_in(g, "q", 5, 6)
            dma_in(g, "c", 6, 7)
            dma_in(g, "c", 7, 8)

            # row 64 is the lhsT of the K=1 Z-broadcast matmul
            ones65 = const.tile([65, 64], bf)
            nc.vector.memset(ones65[:], 1.0)
            # ACT table warmup: preload EXP during startup (table load ~1.3us)
            warm = const.tile([65, 16], bf)
            nc.scalar.activation(warm[:], ones65[:, 0:16], Exp)

            # ---- projection outputs --------------------------------------
            qproj = proj.tile([P, POS], bf)   # Q^T  (2 heads on partitions)
            kproj = proj.tile([P, POS], bf)   # K^T
            # V position-major: per pos-tile [V_h0(64) | 1 | pad | V_h1(64) | 1 | pad]
            v_sb = proj.tile([128, NPT, 2, VW], bf)
            nc.vector.memset(v_sb[:, :, :, 64:65], 1.0)

            def emit_qk(which, ch):
                """Q^T or K^T projection for one 512-pos chunk."""
                src, w_sb, b_sb, dst = (
                    (qt_sb, wq_sb, bq_sb, qproj)
                    if which == "q"
                    else (ct_sb, wk_sb, bk_sb, kproj)
                )
                c0 = ch * QB
                ps = ps_m.tile([128, QB], f32, tag="m", name=f"psqk{which}{ch}")
                for t in range(ET):
                    nc.tensor.matmul(
                        ps[:],
                        w_sb[:, t, :],
                        src[:, t, c0 : c0 + QB],
                        start=(t == 0),
                        stop=(t == ET - 1),
                    )
                nc.vector.tensor_scalar_add(dst[:, c0 : c0 + QB], ps[:], b_sb[:])

            def emit_v(pt):
                """V projection (position-major) for one 128-pos tile."""
                psv = ps_m.tile([128, 128], f32, tag="m", name=f"psv{pt}")
                for t in range(ET):
                    nc.tensor.matmul(
                        psv[:],
                        ct_sb[:, t, pt * 128 : (pt + 1) * 128],
                        wv_sb[:, t, :],
                        start=(t == 0),
                        stop=(t == ET - 1),
                    )
                nc.vector.tensor_add(
                    v_sb[:, pt, :, 0:64],
                    psv.rearrange("p (h d) -> p h d", h=2),
                    bv_sb[:].rearrange("p (h d) -> p h d", h=2),
                )

            # ---- attention unit machinery --------------------------------
            state = {}

            def unit_start(u):
                state[u] = {
                    "atts": [
                        ps_att.tile([65, QB], f32, tag="att", name=f"att{u}{h}")
                        for h in range(2)
                    ],
                    "eg": {},
                }

            def emit_scores_exp(u, ci):
                b = u // 4
                q0 = u * QB
                c0 = b * CL + ci * 128
                sg = ps_s.tile([128, 2 * QB], f32, tag="sg", name=f"sg{u}_{ci}")
                for h in range(2):
                    hp = h * 64
                    nc.tensor.matmul(
                        sg[:, h * QB : (h + 1) * QB],
                        kproj[hp : hp + 64, c0 : c0 + 128],
                        qproj[hp : hp + 64, q0 : q0 + QB],
                        start=True,
                        stop=True,
                    )
                eg = egp.tile([128, 2 * QB], bf, tag="eg", name=f"eg{u}_{ci}")
                nc.scalar.activation(eg[:], sg[:], Exp, scale=0.125)
                state[u]["eg"][ci] = eg

            def emit_attended(u, ci):
                b = u // 4
                eg = state[u]["eg"].pop(ci)
                for h in range(2):
                    nc.tensor.matmul(
                        state[u]["atts"][h][:],
                        v_sb[:, b * CT + ci, h, 0:65],
                        eg[:, h * QB : (h + 1) * QB],
                        start=(ci == 0),
                        stop=(ci == CT - 1),
                    )

            def emit_epi_copy(u):
                """Evacuate attended+Z psum -> sbuf (frees the atts tiles)."""
                st = state[u]
                st["attus"] = []
                for h in range(2):
                    attu = zp.tile([65, QB], bf, tag=f"attu{h}", name=f"attu{u}{h}")
                    nc.vector.tensor_copy(attu[:], st["atts"][h][:])
                    st["attus"].append(attu)

            def emit_epi_norm(u, h1_first=True):
                """Broadcast Z, normalize -> an tile."""
                st = state[u]
                st["an"] = anp.tile([P, QB], bf, tag="an", name=f"an{u}")
                for h in (1, 0) if h1_first else (0, 1):
                    attu = st["attus"][h]
                    zbp = ps_m.tile([64, QB], f32, tag="m", name=f"zbp{u}{h}")
                    nc.tensor.matmul(
                        zbp[:], ones65[64:65, :], attu[64:65, :], start=True, stop=True
                    )
                    ztr = zp.tile([64, QB], f32, tag=f"ztr{h}", name=f"ztr{u}{h}")
                    nc.vector.reciprocal_approx_fast(ztr[:], zbp[:])
                    if h == 0:
                        nc.vector.tensor_tensor(
                            st["an"][0:64, :], attu[0:64, :], ztr[:], op=mult
                        )
                    else:
                        an1 = zp.tile([64, QB], bf, tag="an1", name=f"an1{u}")
                        nc.vector.tensor_tensor(an1[:], attu[0:64, :], ztr[:], op=mult)
                        nc.sync.dma_start(st["an"][64:128, :], an1[:])

            def emit_epi_po(u, eo, pool_eng=False):
                """One e-tile of the output projection for unit u; the 8
                e-tiles accumulate into one SBUF tile, flushed as one DMA."""
                q0 = u * QB
                st = state[u]
                if eo == 0:
                    st["ob"] = obp.tile([128, ET, QB], bf, tag="ob", name=f"ob{u}")
                po = ps_m.tile([128, QB], f32, tag="m", name=f"po{u}{eo}")
                nc.tensor.matmul(
                    po[:],
                    wo_sb[:, eo * 128 : (eo + 1) * 128],
                    st["an"][:],
                    start=True,
                    stop=True,
                )
                if pool_eng:
                    nc.scalar.copy(st["ob"][:, eo, :], po[:])
                else:
                    nc.vector.tensor_copy(st["ob"][:, eo, :], po[:])
                if u == NU - 1:
                    # tail: halves on the idle gpsimd ring (fast,
                    # non-blocking issue) so the last transfer is small
                    if eo == 3:
                        nc.gpsimd.dma_start(
                            outT_r[:, 0:4, q0 : q0 + QB], st["ob"][:, 0:4, :]
                        )
                    elif eo == ET - 1:
                        nc.gpsimd.dma_start(
                            outT_r[:, 4:ET, q0 : q0 + QB], st["ob"][:, 4:ET, :]
                        )
                elif eo == ET - 1:
                    nc.sync.dma_start(
                        outT_r[:, :, q0 : q0 + QB], st["ob"][:, :, :]
                    )

            # ---- PE p-state warmup: back-to-back accumulating matmuls (no
            # inter-MM deps) while the first input chunks stream in, so real
            # projections start at 2.4GHz --
            warm2 = const.tile([128, QB], bf)
            nc.vector.memset(warm2[:], 0.0)
            wps = ps_m.tile([128, QB], f32, tag="m", name="wps")
            for w in range(20):
                nc.tensor.matmul(
                    wps[:], warm2[:, 0:128], warm2[:], start=(w == 0), stop=(w == 19)
                )

            # ---- pre-unit-0 minimal projections --------------------------
            emit_qk("c", 0)      # kproj ctx chunk 0 (b0)
            emit_qk("q", 0)      # qproj q chunk 0 (unit 0)
            emit_v(0)
            emit_v(1)

            # just-in-time extras: extras[u][ci] emitted after that ci's
            # scores/exp/attended and any epilogue piece. Deadlines: kproj
            # ch c before its first consuming ci; vproj pt before its
            # attended (emitted at ci = pt%16 + 1); qproj u+1 before u+1 ci0.
            extras = {u: {ci: [] for ci in range(CT)} for u in range(NU)}

            def sched(u, ci, fn, *a):
                extras[u][ci].append((fn, a))

            # unit 0: rest of b0 K/V proj
            sched(0, 0, emit_v, 2)
            sched(0, 1, emit_v, 3)
            sched(0, 1, emit_qk, "c", 1)
            sched(0, 2, emit_v, 4)
            sched(0, 3, emit_v, 5)
            sched(0, 4, emit_v, 6)
            sched(0, 5, emit_v, 7)
            sched(0, 5, emit_qk, "c", 2)
            sched(0, 6, emit_v, 8)
            sched(0, 7, emit_v, 9)
            sched(0, 8, emit_v, 10)
            sched(0, 9, emit_v, 11)
            sched(0, 9, emit_qk, "c", 3)
            sched(0, 10, emit_v, 12)
            sched(0, 11, emit_v, 13)
            sched(0, 12, emit_v, 14)
            sched(0, 13, emit_v, 15)
            sched(0, 14, emit_qk, "q", 1)
            # unit 1: qproj for unit 2
            sched(1, 13, emit_qk, "q", 2)
            # unit 2: start b1 context work + qproj(3)
            sched(2, 10, emit_qk, "c", 4)
            sched(2, 11, emit_v, 16)
            sched(2, 12, emit_v, 17)
            sched(2, 13, emit_qk, "q", 3)
            sched(2, 14, emit_v, 18)
            sched(2, 15, emit_v, 19)
            # unit 3: more b1 + qproj(4)
            sched(3, 10, emit_qk, "c", 5)
            sched(3, 11, emit_v, 20)
            sched(3, 12, emit_v, 21)
            sched(3, 13, emit_qk, "c", 6)
            sched(3, 14, emit_v, 22)
            sched(3, 15, emit_qk, "q", 4)
            # unit 4 (b1): remaining b1 vproj just-in-time
            sched(4, 0, emit_v, 23)
            sched(4, 1, emit_v, 24)
            sched(4, 2, emit_v, 25)
            sched(4, 3, emit_v, 26)
            sched(4, 4, emit_v, 27)
            sched(4, 4, emit_qk, "c", 7)
            sched(4, 5, emit_v, 28)
            sched(4, 6, emit_v, 29)
            sched(4, 7, emit_v, 30)
            sched(4, 8, emit_v, 31)
            sched(4, 10, emit_qk, "q", 5)
            sched(5, 10, emit_qk, "q", 6)
            sched(6, 10, emit_qk, "q", 7)

            # ---- main loop: 8 units, software-pipelined across unit
            # boundaries (attended(u-1,15) + epilogue emitted inside unit u)
            for u in range(NU):
                unit_start(u)
                for ci in range(CT):
                    emit_scores_exp(u, ci)
                    if ci >= 1:
                        emit_attended(u, ci - 1)
                    elif u > 0:
                        emit_attended(u - 1, CT - 1)
                        emit_epi_copy(u - 1)
                    if u > 0:
                        if ci == 1:
                            emit_epi_norm(u - 1)
                        elif 2 <= ci <= 9:
                            emit_epi_po(u - 1, ci - 2)
                    for fn, a in extras[u][ci]:
                        fn(*a)
            emit_attended(NU - 1, CT - 1)
            emit_epi_copy(NU - 1)
            emit_epi_norm(NU - 1)
            for eo in range(ET):
                emit_epi_po(NU - 1, eo, pool_eng=(eo % 2 == 1))

    nc.compile()
    return nc


def get_nc():
    if "nc" not in _CACHE:
        _CACHE["nc"] = _build_nc()
    return _CACHE["nc"]


def make_in_maps(query, context, Wq, bq, Wk, bk, Wv, bv, Wo, bo):
    qT = np.asarray(query, np.float32).reshape(POS, E).T.astype(BF16)
    cT = np.asarray(context, np.float32).reshape(POS, E).T.astype(BF16)
    in_maps = []
    for c in range(NCORES):
        F = slice(P * c, P * (c + 1))
        in_maps.append(
            {
                "qT": qT,
                "cT": cT,
                "wq": np.ascontiguousarray(Wq[:, F]).astype(BF16),
                "wk": np.ascontiguousarray(Wk[:, F]).astype(BF16),
                "wv": np.ascontiguousarray(Wv[:, F]).astype(BF16),
                "wo": np.ascontiguousarray(Wo[F, :]).astype(BF16),
                "bq": np.ascontiguousarray(bq[F]).reshape(P, 1).astype(np.float32),
                "bk": np.ascontiguousarray(bk[F]).reshape(P, 1).astype(np.float32),
                "bvt": np.ascontiguousarray(
                    np.broadcast_to(bv[F], (128, P))
                ).astype(np.float32),
            }
        )
    return in_maps


def assemble_output(partials, bo):
    total = np.zeros((E, POS), np.float32)
    for p in partials:
        total += p
    out = total.T.reshape(B, QL, E) + np.asarray(bo, np.float32)
    return out.astype(np.float32)


def kernel(query, context, Wq, bq, Wk, bk, Wv, bv, Wo, bo):
    from concourse import bass_utils

    nc = get_nc()
    in_maps = make_in_maps(query, context, Wq, bq, Wk, bk, Wv, bv, Wo, bo)
    res = bass_utils.run_bass_kernel_spmd(nc, in_maps, core_ids=list(range(NCORES)))
    partials = [res.results[c]["outT"] for c in range(NCORES)]
    return assemble_output(partials, bo)



# revision 51
# speedup vs baseline: 1.0142x; 1.0092x over previous
"""Cross-attention layer (B=2, QL=CL=2048, E=1024, 16 heads x 64d) on 8 TRN2 cores.

Sharding: tensor-parallel over heads. Core c owns heads (2c, 2c+1), i.e. a
128-wide feature slice of Wq/Wk/Wv columns and Wo rows. Each core computes a
full-shape partial of the output projection; the host sums the 8 partials and
adds bo.

v4: all math stays bf16 (attention amplifies per-element quantization 1:1 —
fp8 anywhere on the Q/K/V/eg path costs 2-7% output error vs the 2% budget).
Wins over v3 come from the measured DMA model (3 dynamic queues; gpsimd's
swDGE issues don't block on the transfer and its ring is several times
faster than the sync/scalar hw rings once compute runs; ~10 completion sems
recycle per pool in emission order):
  - the whole input stream rides the gpsimd ring as one DMA per logical
    tensor/chunk, in strict deadline order, with only q0/q1 on the sync
    ring; an/ob DMAs get sync's fresh sem pool so they never stall on
    recycled claims.
  - PE warmup is a single accumulation group (back-to-back, no psum-pool
    WAR chain) bridging the framework preamble to the first chunk arrival.
  - attended(u,15) + the unit epilogue are emitted inside unit u+1's ci=0
    slot (software pipeline across unit boundaries); the 8 output-projection
    e-tiles accumulate into one SBUF tile flushed as a single DMA per unit;
    the final unit's psum->sbuf casts alternate DVE/ACT.
Scores use the 2-head row-tiling trick (K=64 pairs execute concurrently in
different PE row groups); softmax skips max-subtraction (scores ~ N(0,1)
after the 1/8 scale) and Z comes from a ones column appended to V.
Steady state is PE-bound (~193us PE busy/core: QKV+O projections 70us,
scores+attended+Z 118us) with ACT exp at ~142us; both run >90% occupancy
mid-kernel.
"""

import numpy as np
import ml_dtypes

E = 1024          # embed dim
H = 16            # heads
D = 64            # head dim
B = 2
QL = CL = 2048
POS = B * QL      # 4096 flattened positions
NCORES = 8
P = 128           # per-core feature slice (2 heads x 64)
ET = E // 128     # 8 contraction e-tiles
NPT = POS // 128  # 32 position tiles (V)
CT = CL // 128    # 16 context tiles per batch
QB = 512          # q-block (free dim of attention matmuls)
NU = POS // QB    # 8 units (b, qb)
VW = 66           # per-head stride in v_sb: 64 V cols + 1 ones + 1 pad

BF16 = ml_dtypes.bfloat16

_CACHE = {}


def _build_nc():
    import concourse.bacc as bacc
    import concourse.mybir as mybir
    import concourse.tile as tile

    bf = mybir.dt.bfloat16
    f32 = mybir.dt.float32
    Exp = mybir.ActivationFunctionType.Exp
    mult = mybir.AluOpType.mult

    nc = bacc.Bacc(
        "TRN2",
        target_bir_lowering=False,
        debug=False,
        enable_asserts=False,
        num_devices=NCORES,
    )

    qT_d = nc.dram_tensor("qT", [E, POS], bf, kind="ExternalInput").ap()
    cT_d = nc.dram_tensor("cT", [E, POS], bf, kind="ExternalInput").ap()
    wq_d = nc.dram_tensor("wq", [E, P], bf, kind="ExternalInput").ap()
    wk_d = nc.dram_tensor("wk", [E, P], bf, kind="ExternalInput").ap()
    wv_d = nc.dram_tensor("wv", [E, P], bf, kind="ExternalInput").ap()
    wo_d = nc.dram_tensor("wo", [P, E], bf, kind="ExternalInput").ap()
    bq_d = nc.dram_tensor("bq", [P, 1], f32, kind="ExternalInput").ap()
    bk_d = nc.dram_tensor("bk", [P, 1], f32, kind="ExternalInput").ap()
    bv_d = nc.dram_tensor("bvt", [128, P], f32, kind="ExternalInput").ap()
    outT_d = nc.dram_tensor("outT", [E, POS], bf, kind="ExternalOutput").ap()
    outT_r = outT_d.rearrange("(t p) m -> p t m", p=128)

    with tile.TileContext(nc) as tc:
        with (
            tc.tile_pool(name="const", bufs=1) as const,
            tc.tile_pool(name="inp", bufs=1) as inp,
            tc.tile_pool(name="proj", bufs=1) as proj,
            tc.tile_pool(name="egp", bufs=3) as egp,
            tc.tile_pool(name="zp", bufs=2) as zp,
            tc.tile_pool(name="anp", bufs=2) as anp,
            tc.tile_pool(name="obp", bufs=2) as obp,
            tc.tile_pool(name="ps_s", bufs=2, space="PSUM") as ps_s,
            tc.tile_pool(name="ps_att", bufs=2, space="PSUM") as ps_att,
            tc.tile_pool(name="ps_m", bufs=2, space="PSUM") as ps_m,
        ):
            # ---- weights needed first, then the first input chunks -------
            wk_sb = const.tile([128, ET, P], bf)
            wq_sb = const.tile([128, ET, P], bf)
            wv_sb = const.tile([128, ET, P], bf)
            bq_sb = const.tile([P, 1], f32)
            bk_sb = const.tile([P, 1], f32)

            qt_sb = inp.tile([128, ET, POS], bf)
            ct_sb = inp.tile([128, ET, POS], bf)
            qT_r = qT_d.rearrange("(t p) m -> p t m", p=128)
            cT_r = cT_d.rearrange("(t p) m -> p t m", p=128)

            def dma_in(eng, which, ch0, ch1, t0=0, t1=ET):
                """Load chunks [ch0, ch1) of qT/cT (e-tiles [t0,t1), one DMA)."""
                src, dst = (qT_r, qt_sb) if which == "q" else (cT_r, ct_sb)
                c0, c1 = ch0 * QB, ch1 * QB
                eng.dma_start(dst[:, t0:t1, c0:c1], src[:, t0:t1, c0:c1])

            wo_sb = const.tile([P, E], bf)
            bv_sb = const.tile([128, P], f32)
            # DMA model: 3 dynamic queues.  gpsimd's (Q0) is the fastest and
            # its issue instructions do NOT block on the transfer (software
            # DGE), while sync/scalar hw queues get starved when Q0 is
            # active and their issues occupy the queue until completion.  So
            # the whole input stream goes on gpsimd, in strict priority
            # order (issue rate ~0.7-1.1us gates the head of the stream).
            # sync keeps its own sem pool free of recycling for the an/ob
            # DMAs; scalar stays pure-ACT.
            g = nc.gpsimd
            g.dma_start(wk_sb[:], wk_d.rearrange("(t p) m -> p t m", p=128))
            dma_in(g, "c", 0, 1, 0, 4)
            dma_in(g, "c", 0, 1, 4, 8)
            g.dma_start(wq_sb[:], wq_d.rearrange("(t p) m -> p t m", p=128))
            g.dma_start(bk_sb[:], bk_d[:])
            g.dma_start(bq_sb[:], bq_d[:])
            g.dma_start(wv_sb[:], wv_d.rearrange("(t p) m -> p t m", p=128))
            g.dma_start(bv_sb[:], bv_d[:])
            # q chunks 0-1 on the sync hw ring, concurrent with gpsimd's
            dma_in(nc.sync, "q", 0, 1, 0, 4)
            dma_in(nc.sync, "q", 0, 1, 4, 8)
            dma_in(nc.sync, "q", 1, 2)
            dma_in(nc.scalar, "c", 2, 3)
            dma_in(nc.scalar, "q", 6, 8)
            dma_in(g, "c", 1, 2)
            g.dma_start(wo_sb[:], wo_d[:])
            dma_in(g, "c", 3, 4)
            dma_in(g, "q", 2, 3)
            dma_in(g, "q", 3, 4)
            dma_in(g, "c", 4, 6)
            dma_in(g, "q", 4, 5)
            dma_in(g, "q", 5, 6)
            dma_in(g, "c", 6, 7)
            dma_in(g, "c", 7, 8)

            # row 64 is the lhsT of the K=1 Z-broadcast matmul
            ones65 = const.tile([65, 64], bf)
            nc.vector.memset(ones65[:], 1.0)
            # ACT table warmup: preload EXP during startup (table load ~1.3us)
            warm = const.tile([65, 16], bf)
            nc.scalar.activation(warm[:], ones65[:, 0:16], Exp)

            # ---- projection outputs --------------------------------------
            qproj = proj.tile([P, POS], bf)   # Q^T  (2 heads on partitions)
            kproj = proj.tile([P, POS], bf)   # K^T
            # V position-major: per pos-tile [V_h0(64) | 1 | pad | V_h1(64) | 1 | pad]
            v_sb = proj.tile([128, NPT, 2, VW], bf)
            nc.vector.memset(v_sb[:, :, :, 64:65], 1.0)

            def emit_qk(which, ch):
                """Q^T or K^T projection for one 512-pos chunk."""
                src, w_sb, b_sb, dst = (
                    (qt_sb, wq_sb, bq_sb, qproj)
                    if which == "q"
                    else (ct_sb, wk_sb, bk_sb, kproj)
                )
                c0 = ch * QB
                ps = ps_m.tile([128, QB], f32, tag="m", name=f"psqk{which}{ch}")
                for t in range(ET):
                    nc.tensor.matmul(
                        ps[:],
                        w_sb[:, t, :],
                        src[:, t, c0 : c0 + QB],
                        start=(t == 0),
                        stop=(t == ET - 1),
                    )
                nc.vector.tensor_scalar_add(dst[:, c0 : c0 + QB], ps[:], b_sb[:])

            def emit_v(pt):
                """V projection (position-major) for one 128-pos tile."""
                psv = ps_m.tile([128, 128], f32, tag="m", name=f"psv{pt}")
                for t in range(ET):
                    nc.tensor.matmul(
                        psv[:],
                        ct_sb[:, t, pt * 128 : (pt + 1) * 128],
                        wv_sb[:, t, :],
                        start=(t == 0),
                        stop=(t == ET - 1),
                    )
                nc.vector.tensor_add(
                    v_sb[:, pt, :, 0:64],
                    psv.rearrange("p (h d) -> p h d", h=2),
                    bv_sb[:].rearrange("p (h d) -> p h d", h=2),
                )

            # ---- attention unit machinery --------------------------------
            state = {}

            def unit_start(u):
                state[u] = {
                    "atts": [
                        ps_att.tile([65, QB], f32, tag="att", name=f"att{u}{h}")
                        for h in range(2)
                    ],
                    "eg": {},
                }

            def emit_scores_exp(u, ci):
                b = u // 4
                q0 = u * QB
                c0 = b * CL + ci * 128
                sg = ps_s.tile([128, 2 * QB], f32, tag="sg", name=f"sg{u}_{ci}")
                for h in range(2):
                    hp = h * 64
                    nc.tensor.matmul(
                        sg[:, h * QB : (h + 1) * QB],
                        kproj[hp : hp + 64, c0 : c0 + 128],
                        qproj[hp : hp + 64, q0 : q0 + QB],
                        start=True,
                        stop=True,
                    )
                eg = egp.tile([128, 2 * QB], bf, tag="eg", name=f"eg{u}_{ci}")
                nc.scalar.activation(eg[:], sg[:], Exp, scale=0.125)
                state[u]["eg"][ci] = eg

            def emit_attended(u, ci):
                b = u // 4
                eg = state[u]["eg"].pop(ci)
                for h in range(2):
                    nc.tensor.matmul(
                        state[u]["atts"][h][:],
                        v_sb[:, b * CT + ci, h, 0:65],
                        eg[:, h * QB : (h + 1) * QB],
                        start=(ci == 0),
                        stop=(ci == CT - 1),
                    )

            def emit_epi_copy(u):
                """Evacuate attended+Z psum -> sbuf (frees the atts tiles)."""
                st = state[u]
                st["attus"] = []
                for h in range(2):
                    attu = zp.tile([65, QB], bf, tag=f"attu{h}", name=f"attu{u}{h}")
                    nc.vector.tensor_copy(attu[:], st["atts"][h][:])
                    st["attus"].append(attu)

            def emit_epi_norm(u, h1_first=True):
                """Broadcast Z, normalize -> an tile."""
                st = state[u]
                st["an"] = anp.tile([P, QB], bf, tag="an", name=f"an{u}")
                for h in (1, 0) if h1_first else (0, 1):
                    attu = st["attus"][h]
                    zbp = ps_m.tile([64, QB], f32, tag="m", name=f"zbp{u}{h}")
                    nc.tensor.matmul(
                        zbp[:], ones65[64:65, :], attu[64:65, :], start=True, stop=True
                    )
                    ztr = zp.tile([64, QB], f32, tag=f"ztr{h}", name=f"ztr{u}{h}")
                    nc.vector.reciprocal_approx_fast(ztr[:], zbp[:])
                    if h == 0:
                        nc.vector.tensor_tensor(
                            st["an"][0:64, :], attu[0:64, :], ztr[:], op=mult
                        )
                    else:
                        an1 = zp.tile([64, QB], bf, tag="an1", name=f"an1{u}")
                        nc.vector.tensor_tensor(an1[:], attu[0:64, :], ztr[:], op=mult)
                        nc.sync.dma_start(st["an"][64:128, :], an1[:])

            def emit_epi_po(u, eo, pool_eng=False):
                """One e-tile of the output projection for unit u; the 8
                e-tiles accumulate into one SBUF tile, flushed as one DMA."""
                q0 = u * QB
                st = state[u]
                if eo == 0:
                    st["ob"] = obp.tile([128, ET, QB], bf, tag="ob", name=f"ob{u}")
                if u == NU - 1:
                    # tail: sg/att banks are free once the last exp and
                    # attended drain, so round-robin the po tiles over all
                    # three psum pools — the 8 matmuls then don't serialize
                    # on the cast WAR chain of the 2-buffer ps_m pool
                    poolL, tagL = [(ps_m, "m"), (ps_s, "sg"), (ps_att, "att")][eo % 3]
                    po = poolL.tile([128, QB], f32, tag=tagL, name=f"po{u}{eo}")
                else:
                    po = ps_m.tile([128, QB], f32, tag="m", name=f"po{u}{eo}")
                nc.tensor.matmul(
                    po[:],
                    wo_sb[:, eo * 128 : (eo + 1) * 128],
                    st["an"][:],
                    start=True,
                    stop=True,
                )
                if pool_eng:
                    nc.scalar.copy(st["ob"][:, eo, :], po[:])
                else:
                    nc.vector.tensor_copy(st["ob"][:, eo, :], po[:])
                if u == NU - 1:
                    # tail: halves on the idle gpsimd ring (fast,
                    # non-blocking issue) so the last transfer is small
                    if eo == 3:
                        nc.gpsimd.dma_start(
                            outT_r[:, 0:4, q0 : q0 + QB], st["ob"][:, 0:4, :]
                        )
                    elif eo == ET - 1:
                        nc.gpsimd.dma_start(
                            outT_r[:, 4:ET, q0 : q0 + QB], st["ob"][:, 4:ET, :]
                        )
                elif eo == ET - 1:
                    nc.sync.dma_start(
                        outT_r[:, :, q0 : q0 + QB], st["ob"][:, :, :]
                    )

            # ---- PE p-state warmup: back-to-back accumulating matmuls (no
            # inter-MM deps) while the first input chunks stream in, so real
            # projections start at 2.4GHz --
            warm2 = const.tile([128, QB], bf)
            nc.vector.memset(warm2[:], 0.0)
            wps = ps_m.tile([128, QB], f32, tag="m", name="wps")
            for w in range(20):
                nc.tensor.matmul(
                    wps[:], warm2[:, 0:128], warm2[:], start=(w == 0), stop=(w == 19)
                )

            # ---- pre-unit-0 minimal projections --------------------------
            emit_qk("c", 0)      # kproj ctx chunk 0 (b0)
            emit_qk("q", 0)      # qproj q chunk 0 (unit 0)
            emit_v(0)
            emit_v(1)

            # just-in-time extras: extras[u][ci] emitted after that ci's
            # scores/exp/attended and any epilogue piece. Deadlines: kproj
            # ch c before its first consuming ci; vproj pt before its
            # attended (emitted at ci = pt%16 + 1); qproj u+1 before u+1 ci0.
            extras = {u: {ci: [] for ci in range(CT)} for u in range(NU)}

            def sched(u, ci, fn, *a):
                extras[u][ci].append((fn, a))

            # unit 0: rest of b0 K/V proj
            sched(0, 0, emit_v, 2)
            sched(0, 1, emit_v, 3)
            sched(0, 1, emit_qk, "c", 1)
            sched(0, 2, emit_v, 4)
            sched(0, 3, emit_v, 5)
            sched(0, 4, emit_v, 6)
            sched(0, 5, emit_v, 7)
            sched(0, 5, emit_qk, "c", 2)
            sched(0, 6, emit_v, 8)
            sched(0, 7, emit_v, 9)
            sched(0, 8, emit_v, 10)
            sched(0, 9, emit_v, 11)
            sched(0, 9, emit_qk, "c", 3)
            sched(0, 10, emit_v, 12)
            sched(0, 11, emit_v, 13)
            sched(0, 12, emit_v, 14)
            sched(0, 13, emit_v, 15)
            sched(0, 14, emit_qk, "q", 1)
            # unit 1: qproj for unit 2
            sched(1, 13, emit_qk, "q", 2)
            # unit 2: start b1 context work + qproj(3)
            sched(2, 10, emit_qk, "c", 4)
            sched(2, 11, emit_v, 16)
            sched(2, 12, emit_v, 17)
            sched(2, 13, emit_qk, "q", 3)
            sched(2, 14, emit_v, 18)
            sched(2, 15, emit_v, 19)
            # unit 3: more b1 + qproj(4)
            sched(3, 10, emit_qk, "c", 5)
            sched(3, 11, emit_v, 20)
            sched(3, 12, emit_v, 21)
            sched(3, 13, emit_qk, "c", 6)
            sched(3, 14, emit_v, 22)
            sched(3, 15, emit_qk, "q", 4)
            # unit 4 (b1): remaining b1 vproj just-in-time
            sched(4, 0, emit_v, 23)
            sched(4, 1, emit_v, 24)
            sched(4, 2, emit_v, 25)
            sched(4, 3, emit_v, 26)
            sched(4, 4, emit_v, 27)
            sched(4, 4, emit_qk, "c", 7)
            sched(4, 5, emit_v, 28)
            sched(4, 6, emit_v, 29)
            sched(4, 7, emit_v, 30)
            sched(4, 8, emit_v, 31)
            sched(4, 10, emit_qk, "q", 5)
            sched(5, 10, emit_qk, "q", 6)
            sched(6, 10, emit_qk, "q", 7)

            # ---- main loop: 8 units, software-pipelined across unit
            # boundaries (attended(u-1,15) + epilogue emitted inside unit u)
            for u in range(NU):
                unit_start(u)
                for ci in range(CT):
                    emit_scores_exp(u, ci)
                    if ci >= 1:
                        emit_attended(u, ci - 1)
                    elif u > 0:
                        emit_attended(u - 1, CT - 1)
                        emit_epi_copy(u - 1)
                    if u > 0:
                        if ci == 1:
                            emit_epi_norm(u - 1)
                        elif 2 <= ci <= 9:
                            emit_epi_po(u - 1, ci - 2)
                    for fn, a in extras[u][ci]:
                        fn(*a)
            emit_attended(NU - 1, CT - 1)
            emit_epi_copy(NU - 1)
            emit_epi_norm(NU - 1)
            for eo in range(ET):
                emit_epi_po(NU - 1, eo, pool_eng=(eo % 2 == 1))

    nc.compile()
    return nc


def get_nc():
    if "nc" not in _CACHE:
        _CACHE["nc"] = _build_nc()
    return _CACHE["nc"]


def make_in_maps(query, context, Wq, bq, Wk, bk, Wv, bv, Wo, bo):
    qT = np.asarray(query, np.float32).reshape(POS, E).T.astype(BF16)
    cT = np.asarray(context, np.float32).reshape(POS, E).T.astype(BF16)
    in_maps = []
    for c in range(NCORES):
        F = slice(P * c, P * (c + 1))
        in_maps.append(
            {
                "qT": qT,
                "cT": cT,
                "wq": np.ascontiguousarray(Wq[:, F]).astype(BF16),
                "wk": np.ascontiguousarray(Wk[:, F]).astype(BF16),
                "wv": np.ascontiguousarray(Wv[:, F]).astype(BF16),
                "wo": np.ascontiguousarray(Wo[F, :]).astype(BF16),
                "bq": np.ascontiguousarray(bq[F]).reshape(P, 1).astype(np.float32),
                "bk": np.ascontiguousarray(bk[F]).reshape(P, 1).astype(np.float32),
                "bvt": np.ascontiguousarray(
                    np.broadcast_to(bv[F], (128, P))
                ).astype(np.float32),
            }
        )
    return in_maps


def assemble_output(partials, bo):
    total = np.zeros((E, POS), np.float32)
    for p in partials:
        total += p
    out = total.T.reshape(B, QL, E) + np.asarray(bo, np.float32)
    return out.astype(np.float32)


def kernel(query, context, Wq, bq, Wk, bk, Wv, bv, Wo, bo):
    from concourse import bass_utils

    nc = get_nc()
    in_maps = make_in_maps(query, context, Wq, bq, Wk, bk, Wv, bv, Wo, bo)
    res = bass_utils.run_bass_kernel_spmd(nc, in_maps, core_ids=list(range(NCORES)))
    partials = [res.results[c]["outT"] for c in range(NCORES)]
    return assemble_output(partials, bo)

